# revision 4
# baseline (speedup 1.0000x reference)
"""Debayer 3x3 kernel for Trainium2 (Bass/Tile), batch-sharded over 8 NeuronCores.

Reference semantics: 1->5 channel 3x3 conv (identity, plus-4, diag-4,
horiz-2, vert-2) over an edge-padded Bayer frame, then per-2x2-parity
channel select into RGB.

Strategy (v2: packed byte-lane SIMD, ~2x over the fp16/u8 v1):
  * Identity channel never touches the device (host fills from f32 x).
    Device ships the two non-trivial values per pixel as planes
    A (non-id R-or-G) and B2 (non-id B-or-G), quarter layout:
      A:  (e,e)=c1 (e,o)=c3 (o,e)=c4 (o,o)=c2
      B2: (e,e)=c2 (e,o)=c4 (o,e)=c3 (o,o)=c1
  * PACKING: the image is split into left half (cols 0..959) and right
    half (960..1919); host quantizes q = rint(63*x) (max err 1/126 ~
    7.9e-3, gate is 2e-2) and packs one uint16 word per packed-column:
      word = q[r, c] + 256*q[r, c+960]
    Every engine op is uint16 and processes TWO pixels per element.
    Byte lanes never carry: all intermediate/final lane values <= 252
    (SQ,VQ <= 126; c* <= 252), words <= 64764 < 2^16, and Act's
    internal fp32 is exact for these integers.
  * The four c values are integer sums of q (q-scale 63 => outputs are
    exactly 252*c, still byte-sized):
      SQ = q[l]+q[r]  VQ = q[u]+q[d]
      c1*252 = SQ+VQ   c2*252 = SQ[u]+SQ[d]   c3*252 = 2*SQ   c4*252 = 2*VQ
    The uint16 adds write the output tile DIRECTLY (the u16 word's two
    bytes ARE the two final u8 outputs) -- no cast stage at all, which
    is what made v1 Act/DVE-bound.
  * Engine split: DVE does SQ, VQ, c1, c2 (adds, 2x 16-bit mode,
    ~24.7k el/lane ~ 13.3us); Act does the c3/c4 scale-by-2 muls
    (~8.2k el ~ 7.2us). Equal-shape quarter pairs are fused into single
    instructions via hand-built access patterns (partition dim + a
    stride-delta pair dim), same trick as v1.

Device layout: 128 partitions = 8 packed-col-groups x 16 row-bands:
  partition p = 16*g + b
  band b  -> image rows [68b, 68b+68)       (patch has +-1 halo rows)
  group g -> packed cols [120g, 120g+120)   (patch has +-1 halo cols)
Patch cols stored parity-deinterleaved: [70, 2, 61]; patch col k = 2v+s
corresponds to packed col 120g-1+k, so out col j maps to k=j+1 and
horizontal neighbor sums stay parity-pure and stride-1 (2x DVE mode).
Packing note: packed col -1 is (qpad[:,0], qpad[:,960]) etc -- both
byte lanes are windows of the same edge-padded row, so one u16 plane
pk[r, m] = qpad[r, m] + 256*qpad[r, m+960], m in 0..961, serves all
groups including halos.
"""

import numpy as np

H, W = 1088, 1920
PW = 960         # packed width (byte lanes: +0 and +960)
NB = 16          # row bands per col-group
BH = 68          # output rows per band
NQ = 8           # packed col groups
SW = 120         # packed out cols per group
HSW = SW // 2    # 60 per col-parity
PR = BH + 2      # patch rows (with halo)
HPC = HSW + 1    # 61 patch cols per parity (122 total, with halo)

QLEV = np.float32(63.0)   # quantization levels; device outputs 252*c

_NC_CACHE = {}
LAST_RESULTS = None


def _build(reps=1, *, in_bufs=3, mid_bufs=2, out_bufs=2, out_split=1,
           no_compute=False, out_small=False, **_ignored):
    """Build the Bass module. reps>1 repeats the whole pipeline (bench only).
    Probe flags (bench only): no_compute = DMA skeleton; out_small = full
    compute but 1-row output DMA. out_split: output DMA instruction count."""
    key = (reps, in_bufs, mid_bufs, out_bufs, out_split, no_compute, out_small)
    if key in _NC_CACHE:
        return _NC_CACHE[key]
    import concourse.bacc as bacc
    import concourse.mybir as mybir
    import concourse.tile as tile
    from concourse._compat import get_trn_type
    from concourse.ap import AP

    def merge_pair(a, b):
        # one AP covering two equal-shape quarters: insert a pair dim
        # (stride = offset delta) after the partition dim
        da = int(b.offset) - int(a.offset)
        assert da != 0 and [list(x) for x in a.ap] == [list(x) for x in b.ap]
        dims = [list(a.ap[0])] + [[da, 2]] + [list(x) for x in a.ap[1:]]
        return AP(a.tensor, a.offset, dims)

    u16 = mybir.dt.uint16
    nc = bacc.Bacc(get_trn_type() or "TRN2", target_bir_lowering=False, debug=False)
    xin = nc.dram_tensor("xprep", [128, PR, 2, HPC], u16, kind="ExternalInput")
    yout = nc.dram_tensor("yout", [128, 2, BH, 2, HSW], u16, kind="ExternalOutput")
    # bench-only: earlier reps dump to internal scratch so no two reps write
    # the same DRAM (WAW races hang the exec unit)
    ydumps = [
        nc.dram_tensor(f"ydump{r}", [128, 2, BH, 2, HSW], u16, kind="Internal")
        for r in range(reps - 1)
    ]

    # out-row parity slices (within BH output rows)
    E_, O_ = slice(0, BH, 2), slice(1, BH, 2)
    # patch-row slice for out rows of given parity (out row i -> patch row i+1)
    pE, pO = slice(1, PR - 1, 2), slice(2, PR, 2)
    # SQ rows for diag channel: out row i needs patch rows i and i+2
    dE0, dE1 = slice(0, PR - 2, 2), slice(2, PR, 2)      # even out rows
    dO0, dO1 = slice(1, PR - 1, 2), slice(3, PR, 2)      # odd out rows

    with tile.TileContext(nc) as tc:
        with tc.tile_pool(name="pin", bufs=in_bufs) as pin, \
             tc.tile_pool(name="pmid", bufs=mid_bufs) as pmid, \
             tc.tile_pool(name="pout", bufs=out_bufs) as pout:

            def load(r):
                t = pin.tile([128, PR, 2, HPC], u16, tag="inp", name=f"inp{r}")
                nc.sync.dma_start(out=t[:], in_=xin[:])
                return t

            cur = load(0)
            for r in range(reps):
                ytgt = yout if r == reps - 1 else ydumps[r]
                nxt = load(r + 1) if r + 1 < reps else None
                X = cur  # [128, PR, 2, HPC], packed q, col-deinterleaved
                if no_compute:
                    Yd = pout.tile([128, 2, BH, 2, HSW], u16, tag="y", name=f"y{r}")
                    nc.scalar.mul(Yd[:, 0, 0:1, 0, :], X[:, 0:1, 0, 0:HSW], 1.0)
                    nc.sync.dma_start(out=ytgt[:], in_=Yd[:])
                    cur = nxt
                    continue
                # SQ[p,pr,c,v] = q horiz-pair sum (packed) at patch row pr
                SQ = pmid.tile([128, PR, 2, HSW], u16, tag="sq", name=f"sq{r}")
                nc.vector.tensor_add(SQ[:], X[:, :, :, 0:HSW], X[:, :, :, 1:HPC])
                # VQ[p,i,c,v] = q vert-pair sum at out row i, col parity c
                # (out col 2v -> patch col 2v+1 = odd par; 2v+1 -> even par v+1)
                VQ = pmid.tile([128, BH, 2, HSW], u16, tag="vq", name=f"vq{r}")
                nc.vector.tensor_add(
                    merge_pair(VQ[:, :, 0, :], VQ[:, :, 1, :]),
                    merge_pair(X[:, 0:BH, 1, 0:HSW], X[:, 0:BH, 0, 1:HPC]),
                    merge_pair(X[:, 2:PR, 1, 0:HSW], X[:, 2:PR, 0, 1:HPC]))

                Y = pout.tile([128, 2, BH, 2, HSW], u16, tag="y", name=f"y{r}")
                # c1 = SQ + VQ  at A(e,e) and B2(o,o)  (DVE, direct to Y)
                nc.vector.tensor_add(
                    merge_pair(Y[:, 0, E_, 0, :], Y[:, 1, O_, 1, :]),
                    merge_pair(SQ[:, pE, 0, :], SQ[:, pO, 1, :]),
                    merge_pair(VQ[:, E_, 0, :], VQ[:, O_, 1, :]))
                # c2 = SQ[u] + SQ[d]  at A(o,o) and B2(e,e)
                nc.vector.tensor_add(
                    merge_pair(Y[:, 0, O_, 1, :], Y[:, 1, E_, 0, :]),
                    merge_pair(SQ[:, dO0, 1, :], SQ[:, dE0, 0, :]),
                    merge_pair(SQ[:, dO1, 1, :], SQ[:, dE1, 0, :]))
                # c3 = 2*SQ  at A(e,o) and B2(o,e)  (Act)
                nc.scalar.mul(
                    merge_pair(Y[:, 0, E_, 1, :], Y[:, 1, O_, 0, :]),
                    merge_pair(SQ[:, pE, 1, :], SQ[:, pO, 0, :]), 2.0)
                # c4 = 2*VQ  at A(o,e) and B2(e,o)  (Act)
                nc.scalar.mul(
                    merge_pair(Y[:, 0, O_, 0, :], Y[:, 1, E_, 1, :]),
                    merge_pair(VQ[:, O_, 0, :], VQ[:, E_, 1, :]), 2.0)
                if out_small:
                    nc.sync.dma_start(out=ytgt[:, :, 0:1], in_=Y[:, :, 0:1])
                elif out_split == 1:
                    nc.sync.dma_start(out=ytgt[:], in_=Y[:])
                else:
                    for pl in range(2):
                        nc.sync.dma_start(out=ytgt[:, pl], in_=Y[:, pl])

                cur = nxt

    nc.compile()
    _NC_CACHE[key] = nc
    return nc


def _prep_inputs(x):
    """(B,1,1088,1920) f32 -> (B,128,PR,2,HPC) uint16 packed patches of
    rint(63*x), edge padded, halves packed into byte lanes,
    column-deinterleaved."""
    Bn = x.shape[0]
    q = np.rint(x[:, 0] * QLEV).astype(np.uint8)
    qpad = np.pad(q, ((0, 0), (1, 1), (1, 1)), mode="edge")  # (B,1090,1922)
    pk = (qpad[:, :, 0:PW + 2].astype(np.uint16)
          + (qpad[:, :, PW:PW + 962].astype(np.uint16) << 8))  # (B,1090,962)
    pk = np.ascontiguousarray(pk)
    xprep = np.empty((Bn, 128, PR, 2, HPC), np.uint16)
    st = pk.strides
    for g in range(NQ):
        block = pk[:, :, 120 * g:120 * g + 2 * HPC]
        v = np.lib.stride_tricks.as_strided(
            block, shape=(Bn, NB, PR, 2 * HPC),
            strides=(st[0], BH * st[1], st[1], st[2]))
        xprep[:, g * NB:(g + 1) * NB] = (
            v.reshape(Bn, NB, PR, HPC, 2).transpose(0, 1, 2, 4, 3))
    return xprep


def _assemble(y, x):
    """y (128,2,BH,2,HSW) u16 device planes (252*c packed) + x (1088,1920)
    f32 original -> (3,1088,1920) f32 RGB."""
    AB = np.empty((2, H, W), np.float32)
    lo = (y & np.uint16(255)).astype(np.float32)
    hi = (y >> np.uint16(8)).astype(np.float32)
    for part, c0 in ((lo, 0), (hi, PW)):
        for g in range(NQ):
            blk = part[g * NB:(g + 1) * NB]          # (NB,2,BH,2,HSW)
            for ch in range(2):
                sub = blk[:, ch]                     # (NB,BH,2,HSW)
                AB[ch][:, c0 + SW * g:c0 + SW * (g + 1)] = (
                    sub.transpose(0, 1, 3, 2).reshape(H, SW))
    AB *= np.float32(1.0) / np.float32(252.0)
    A, B2 = AB[0], AB[1]
    out = np.empty((3, H, W), np.float32)
    # R: identity at (e,e), else A
    out[0] = A
    out[0][0::2, 0::2] = x[0::2, 0::2]
    # G: identity at (e,o)/(o,e); c1 from A at (e,e), from B2 at (o,o)
    out[1][0::2, 1::2] = x[0::2, 1::2]
    out[1][1::2, 0::2] = x[1::2, 0::2]
    out[1][0::2, 0::2] = A[0::2, 0::2]
    out[1][1::2, 1::2] = B2[1::2, 1::2]
    # B: identity at (o,o), else B2
    out[2] = B2
    out[2][1::2, 1::2] = x[1::2, 1::2]
    return out


def kernel(x, kernels=None, index=None, **_unused):
    global LAST_RESULTS
    x = np.ascontiguousarray(np.asarray(x), dtype=np.float32)
    Bn = x.shape[0]
    xprep = _prep_inputs(x)
    nc = _build()
    from concourse.bass_utils import run_bass_kernel_spmd
    in_maps = [{"xprep": xprep[i]} for i in range(Bn)]
    res = run_bass_kernel_spmd(nc, in_maps, core_ids=list(range(Bn)))
    LAST_RESULTS = res
    out = np.empty((Bn, 3, H, W), np.float32)
    for i in range(Bn):
        out[i] = _assemble(res.results[i]["yout"], x[i, 0])
    return out


# revision 5
# speedup vs baseline: 1.0012x; 1.0012x over previous
"""Debayer 3x3 kernel for Trainium2 (Bass/Tile), batch-sharded over 8 NeuronCores.

Reference semantics: 1->5 channel 3x3 conv (identity, plus-4, diag-4,
horiz-2, vert-2) over an edge-padded Bayer frame, then per-2x2-parity
channel select into RGB.

Strategy (v2: packed byte-lane SIMD, ~2x over the fp16/u8 v1):
  * Identity channel never touches the device (host fills from f32 x).
    Device ships the two non-trivial values per pixel as planes
    A (non-id R-or-G) and B2 (non-id B-or-G), quarter layout:
      A:  (e,e)=c1 (e,o)=c3 (o,e)=c4 (o,o)=c2
      B2: (e,e)=c2 (e,o)=c4 (o,e)=c3 (o,o)=c1
  * PACKING: the image is split into left half (cols 0..959) and right
    half (960..1919); host quantizes q = rint(63*x) (max err 1/126 ~
    7.9e-3, gate is 2e-2) and packs one uint16 word per packed-column:
      word = q[r, c] + 256*q[r, c+960]
    Every engine op is uint16 and processes TWO pixels per element.
    Byte lanes never carry: all intermediate/final lane values <= 252
    (SQ,VQ <= 126; c* <= 252), words <= 64764 < 2^16, and Act's
    internal fp32 is exact for these integers.
  * The four c values are integer sums of q (q-scale 63 => outputs are
    exactly 252*c, still byte-sized):
      SQ = q[l]+q[r]  VQ = q[u]+q[d]
      c1*252 = SQ+VQ   c2*252 = SQ[u]+SQ[d]   c3*252 = 2*SQ   c4*252 = 2*VQ
    The uint16 adds write the output tile DIRECTLY (the u16 word's two
    bytes ARE the two final u8 outputs) -- no cast stage at all, which
    is what made v1 Act/DVE-bound.
  * Engine split: DVE does SQ, VQ, c1, c2 (adds, 2x 16-bit mode,
    ~24.7k el/lane ~ 13.3us); Act does the c3/c4 scale-by-2 muls
    (~8.2k el ~ 7.2us). Equal-shape quarter pairs are fused into single
    instructions via hand-built access patterns (partition dim + a
    stride-delta pair dim), same trick as v1.

Device layout: 128 partitions = 8 packed-col-groups x 16 row-bands:
  partition p = 16*g + b
  band b  -> image rows [68b, 68b+68)       (patch has +-1 halo rows)
  group g -> packed cols [120g, 120g+120)   (patch has +-1 halo cols)
Patch cols stored parity-deinterleaved: [70, 2, 61]; patch col k = 2v+s
corresponds to packed col 120g-1+k, so out col j maps to k=j+1 and
horizontal neighbor sums stay parity-pure and stride-1 (2x DVE mode).
Packing note: packed col -1 is (qpad[:,0], qpad[:,960]) etc -- both
byte lanes are windows of the same edge-padded row, so one u16 plane
pk[r, m] = qpad[r, m] + 256*qpad[r, m+960], m in 0..961, serves all
groups including halos.
"""

import numpy as np

H, W = 1088, 1920
PW = 960         # packed width (byte lanes: +0 and +960)
NB = 16          # row bands per col-group
BH = 68          # output rows per band
NQ = 8           # packed col groups
SW = 120         # packed out cols per group
HSW = SW // 2    # 60 per col-parity
PR = BH + 2      # patch rows (with halo)
HPC = HSW + 1    # 61 patch cols per parity (122 total, with halo)

QLEV = np.float32(63.0)   # quantization levels; device outputs 252*c

_NC_CACHE = {}
LAST_RESULTS = None


def _build(reps=1, *, in_bufs=2, mid_bufs=2, out_bufs=3, out_split=1,
           no_compute=False, out_small=False, **_ignored):
    """Build the Bass module. reps>1 repeats the whole pipeline (bench only).
    Probe flags (bench only): no_compute = DMA skeleton; out_small = full
    compute but 1-row output DMA. out_split: output DMA instruction count."""
    key = (reps, in_bufs, mid_bufs, out_bufs, out_split, no_compute, out_small)
    if key in _NC_CACHE:
        return _NC_CACHE[key]
    import concourse.bacc as bacc
    import concourse.mybir as mybir
    import concourse.tile as tile
    from concourse._compat import get_trn_type
    from concourse.ap import AP

    def merge_pair(a, b):
        # one AP covering two equal-shape quarters: insert a pair dim
        # (stride = offset delta) after the partition dim
        da = int(b.offset) - int(a.offset)
        assert da != 0 and [list(x) for x in a.ap] == [list(x) for x in b.ap]
        dims = [list(a.ap[0])] + [[da, 2]] + [list(x) for x in a.ap[1:]]
        return AP(a.tensor, a.offset, dims)

    u16 = mybir.dt.uint16
    nc = bacc.Bacc(get_trn_type() or "TRN2", target_bir_lowering=False, debug=False)
    xin = nc.dram_tensor("xprep", [128, PR, 2, HPC], u16, kind="ExternalInput")
    yout = nc.dram_tensor("yout", [128, 2, BH, 2, HSW], u16, kind="ExternalOutput")
    # bench-only: earlier reps dump to internal scratch so no two reps write
    # the same DRAM (WAW races hang the exec unit)
    ydumps = [
        nc.dram_tensor(f"ydump{r}", [128, 2, BH, 2, HSW], u16, kind="Internal")
        for r in range(reps - 1)
    ]

    # out-row parity slices (within BH output rows)
    E_, O_ = slice(0, BH, 2), slice(1, BH, 2)
    # patch-row slice for out rows of given parity (out row i -> patch row i+1)
    pE, pO = slice(1, PR - 1, 2), slice(2, PR, 2)
    # SQ rows for diag channel: out row i needs patch rows i and i+2
    dE0, dE1 = slice(0, PR - 2, 2), slice(2, PR, 2)      # even out rows
    dO0, dO1 = slice(1, PR - 1, 2), slice(3, PR, 2)      # odd out rows

    with tile.TileContext(nc) as tc:
        with tc.tile_pool(name="pin", bufs=in_bufs) as pin, \
             tc.tile_pool(name="pmid", bufs=mid_bufs) as pmid, \
             tc.tile_pool(name="pout", bufs=out_bufs) as pout:

            def load(r):
                t = pin.tile([128, PR, 2, HPC], u16, tag="inp", name=f"inp{r}")
                nc.sync.dma_start(out=t[:], in_=xin[:])
                return t

            cur = load(0)
            for r in range(reps):
                ytgt = yout if r == reps - 1 else ydumps[r]
                nxt = load(r + 1) if r + 1 < reps else None
                X = cur  # [128, PR, 2, HPC], packed q, col-deinterleaved
                if no_compute:
                    Yd = pout.tile([128, 2, BH, 2, HSW], u16, tag="y", name=f"y{r}")
                    nc.scalar.mul(Yd[:, 0, 0:1, 0, :], X[:, 0:1, 0, 0:HSW], 1.0)
                    nc.sync.dma_start(out=ytgt[:], in_=Yd[:])
                    cur = nxt
                    continue
                # SQ[p,pr,c,v] = q horiz-pair sum (packed) at patch row pr
                SQ = pmid.tile([128, PR, 2, HSW], u16, tag="sq", name=f"sq{r}")
                nc.vector.tensor_add(SQ[:], X[:, :, :, 0:HSW], X[:, :, :, 1:HPC])
                # VQ[p,i,c,v] = q vert-pair sum at out row i, col parity c
                # (out col 2v -> patch col 2v+1 = odd par; 2v+1 -> even par v+1)
                VQ = pmid.tile([128, BH, 2, HSW], u16, tag="vq", name=f"vq{r}")
                nc.vector.tensor_add(
                    merge_pair(VQ[:, :, 0, :], VQ[:, :, 1, :]),
                    merge_pair(X[:, 0:BH, 1, 0:HSW], X[:, 0:BH, 0, 1:HPC]),
                    merge_pair(X[:, 2:PR, 1, 0:HSW], X[:, 2:PR, 0, 1:HPC]))

                Y = pout.tile([128, 2, BH, 2, HSW], u16, tag="y", name=f"y{r}")
                # c1 = SQ + VQ  at A(e,e) and B2(o,o)  (DVE, direct to Y)
                nc.vector.tensor_add(
                    merge_pair(Y[:, 0, E_, 0, :], Y[:, 1, O_, 1, :]),
                    merge_pair(SQ[:, pE, 0, :], SQ[:, pO, 1, :]),
                    merge_pair(VQ[:, E_, 0, :], VQ[:, O_, 1, :]))
                # c2 = SQ[u] + SQ[d]  at A(o,o) and B2(e,e)
                nc.vector.tensor_add(
                    merge_pair(Y[:, 0, O_, 1, :], Y[:, 1, E_, 0, :]),
                    merge_pair(SQ[:, dO0, 1, :], SQ[:, dE0, 0, :]),
                    merge_pair(SQ[:, dO1, 1, :], SQ[:, dE1, 0, :]))
                # c3 = 2*SQ  at A(e,o) and B2(o,e)  (Act)
                nc.scalar.mul(
                    merge_pair(Y[:, 0, E_, 1, :], Y[:, 1, O_, 0, :]),
                    merge_pair(SQ[:, pE, 1, :], SQ[:, pO, 0, :]), 2.0)
                # c4 = 2*VQ  at A(o,e) and B2(e,o)  (Act)
                nc.scalar.mul(
                    merge_pair(Y[:, 0, O_, 0, :], Y[:, 1, E_, 1, :]),
                    merge_pair(VQ[:, O_, 0, :], VQ[:, E_, 1, :]), 2.0)
                if out_small:
                    nc.sync.dma_start(out=ytgt[:, :, 0:1], in_=Y[:, :, 0:1])
                elif out_split == 1:
                    nc.sync.dma_start(out=ytgt[:], in_=Y[:])
                else:
                    for pl in range(2):
                        nc.sync.dma_start(out=ytgt[:, pl], in_=Y[:, pl])

                cur = nxt

    nc.compile()
    _NC_CACHE[key] = nc
    return nc


def _prep_inputs(x):
    """(B,1,1088,1920) f32 -> (B,128,PR,2,HPC) uint16 packed patches of
    rint(63*x), edge padded, halves packed into byte lanes,
    column-deinterleaved."""
    Bn = x.shape[0]
    q = np.rint(x[:, 0] * QLEV).astype(np.uint8)
    qpad = np.pad(q, ((0, 0), (1, 1), (1, 1)), mode="edge")  # (B,1090,1922)
    pk = (qpad[:, :, 0:PW + 2].astype(np.uint16)
          + (qpad[:, :, PW:PW + 962].astype(np.uint16) << 8))  # (B,1090,962)
    pk = np.ascontiguousarray(pk)
    xprep = np.empty((Bn, 128, PR, 2, HPC), np.uint16)
    st = pk.strides
    for g in range(NQ):
        block = pk[:, :, 120 * g:120 * g + 2 * HPC]
        v = np.lib.stride_tricks.as_strided(
            block, shape=(Bn, NB, PR, 2 * HPC),
            strides=(st[0], BH * st[1], st[1], st[2]))
        xprep[:, g * NB:(g + 1) * NB] = (
            v.reshape(Bn, NB, PR, HPC, 2).transpose(0, 1, 2, 4, 3))
    return xprep


def _assemble(y, x):
    """y (128,2,BH,2,HSW) u16 device planes (252*c packed) + x (1088,1920)
    f32 original -> (3,1088,1920) f32 RGB."""
    AB = np.empty((2, H, W), np.float32)
    lo = (y & np.uint16(255)).astype(np.float32)
    hi = (y >> np.uint16(8)).astype(np.float32)
    for part, c0 in ((lo, 0), (hi, PW)):
        for g in range(NQ):
            blk = part[g * NB:(g + 1) * NB]          # (NB,2,BH,2,HSW)
            for ch in range(2):
                sub = blk[:, ch]                     # (NB,BH,2,HSW)
                AB[ch][:, c0 + SW * g:c0 + SW * (g + 1)] = (
                    sub.transpose(0, 1, 3, 2).reshape(H, SW))
    AB *= np.float32(1.0) / np.float32(252.0)
    A, B2 = AB[0], AB[1]
    out = np.empty((3, H, W), np.float32)
    # R: identity at (e,e), else A
    out[0] = A
    out[0][0::2, 0::2] = x[0::2, 0::2]
    # G: identity at (e,o)/(o,e); c1 from A at (e,e), from B2 at (o,o)
    out[1][0::2, 1::2] = x[0::2, 1::2]
    out[1][1::2, 0::2] = x[1::2, 0::2]
    out[1][0::2, 0::2] = A[0::2, 0::2]
    out[1][1::2, 1::2] = B2[1::2, 1::2]
    # B: identity at (o,o), else B2
    out[2] = B2
    out[2][1::2, 1::2] = x[1::2, 1::2]
    return out


def kernel(x, kernels=None, index=None, **_unused):
    global LAST_RESULTS
    x = np.ascontiguousarray(np.asarray(x), dtype=np.float32)
    Bn = x.shape[0]
    xprep = _prep_inputs(x)
    nc = _build()
    from concourse.bass_utils import run_bass_kernel_spmd
    in_maps = [{"xprep": xprep[i]} for i in range(Bn)]
    res = run_bass_kernel_spmd(nc, in_maps, core_ids=list(range(Bn)))
    LAST_RESULTS = res
    out = np.empty((Bn, 3, H, W), np.float32)
    for i in range(Bn):
        out[i] = _assemble(res.results[i]["yout"], x[i, 0])
    return out
